# revision 7
# baseline (speedup 1.0000x reference)
"""MoE layer (8 experts, top-2) on 8 Trainium2 NeuronCores.

Strategy (expert parallelism, per the sharding hint):
  Host router:   logits = x @ router_w^T in fp32 BLAS plus the reference's
                 softmax/top-2/normalize — 0.1% of the layer's FLOPs of
                 control logic, all heavy math stays on device.
  Host dispatch: tokens gathered per expert (capacity padded) in fp16.
  Device launch: core e holds expert e's weights; computes
                 y = (relu(x @ W1^T)^2 @ W2^T) * w for its gathered tokens.
                 Matmuls run in fp16 with fp32 PSUM accumulation.
  Host combine:  scatter-add of the two expert contributions per token,
                 ascending expert order (same fp32 summation order as the
                 reference loop).

Expert kernel layout (per core, CAP=2176 tokens, chunks of 256):
  Weights live in two resident SBUF tiles (w1 [128, 8*2048], w2
  [128, 16*1024], both fp16) loaded with a handful of wide multi-dim DMA
  descriptors; x chunks load as one descriptor each via einops-rearranged
  access patterns.  fc1 (x^T chunks -> h^T) runs two chunks ahead of fc2
  (h^T -> y) so the tensor engine never waits on the w2 stream; x loads
  self-throttle through a depth-2 tile pool so weight descriptors own the
  DMA wire during warmup.  relu^2 runs DVE (max) -> Act (square), the
  combine weight is fused into the PSUM->SBUF copy on Act, and y flushes
  in 512-wide halves right after each accumulation chain.
"""

import numpy as np

N_EXPERTS = 8
TOP_K = 2
N_EMBD = 1024
EXPERT_DIM = 2048
N_TOKENS = 8192          # 4 * 2048
N_CORES = 8
TOK_PER_CORE = N_TOKENS // N_CORES
CAP = 2176               # per-expert token capacity (17*128; max observed
                         # count is 2175 for the fixed seed). If routing ever
                         # assigns more than CAP tokens to one expert, the
                         # host runs a second expert pass for the overflow
                         # (correct for any input, never triggered here).
TCH = 256                # expert-kernel token chunk

_CACHE = {}


def _build_expert_module(repeat=1, tch=TCH, interleave=2, n_warm=0,
                         ph_bufs=5, py_bufs=3):
    import concourse.bacc as bacc
    import concourse.mybir as mybir
    import concourse.tile as tile

    f32 = mybir.dt.float32
    f16 = mybir.dt.float16
    D = N_EMBD
    F = EXPERT_DIM
    KD = D // 128
    KF = F // 128

    nc = bacc.Bacc("TRN2", target_bir_lowering=False, debug=False,
                   num_devices=N_CORES)
    xT = nc.dram_tensor("xT", [D, CAP], f16, kind="ExternalInput").ap()
    w1T = nc.dram_tensor("w1T", [D, F], f16, kind="ExternalInput").ap()
    w2T = nc.dram_tensor("w2T", [F, D], f16, kind="ExternalInput").ap()
    wv = nc.dram_tensor("wv", [CAP, 1], f32, kind="ExternalInput").ap()
    y = nc.dram_tensor("y", [CAP, D], f32, kind="ExternalOutput").ap()

    # 3D views: [p, k, cols] with dim0 of the DRAM tensor split as (k p)
    xT3 = xT.rearrange("(k p) t -> p k t", p=128)
    w13 = w1T.rearrange("(k p) f -> p k f", p=128)
    w23 = w2T.rearrange("(k p) d -> p k d", p=128)

    chunks = []
    base = 0
    while base < CAP:
        w = min(tch, CAP - base)
        chunks.append((base, w))
        base += w
    NCH = len(chunks)

    with tile.TileContext(nc) as tc:
        with (
            tc.tile_pool(name="wpool", bufs=1) as wpool,
            tc.tile_pool(name="xpool", bufs=2) as xpool,
            tc.tile_pool(name="hpool", bufs=4) as hpool,
            tc.tile_pool(name="rpool", bufs=4) as rpool,
            tc.tile_pool(name="ypool", bufs=3) as ypool,
            tc.tile_pool(name="ph_pool", bufs=ph_bufs, space="PSUM") as ph_pool,
            tc.tile_pool(name="py_pool", bufs=py_bufs, space="PSUM") as py_pool,
        ):
            if n_warm:
                # PE warm-up: garbage matmuls bridge the initial DMA wait so
                # the tensor engine's p-state ramp completes before real work
                warm_in = wpool.tile([128, 128], f16, tag="warm", name="warm")
                nc.vector.memset(warm_in[:], 0.0)
                wps = ph_pool.tile([128, tch], f32, tag="ph", name="warm_ps")
                for i in range(n_warm):
                    nc.tensor.matmul(wps[:, :128], warm_in[:], warm_in[:],
                                     start=True, stop=True)

            wv_tile = wpool.tile([128, CAP // 128], f32, tag="wv", name="wv")
            w1_tile = wpool.tile([128, KD * F], f16, tag="w1", name="w1")
            w1v = w1_tile[:, :].rearrange("p (k f) -> p k f", f=F)
            w2_tile = wpool.tile([128, KF * D], f16, tag="w2", name="w2")
            w2v = w2_tile[:, :].rearrange("p (k d) -> p k d", d=D)

            def load_w1():
                for q in range(8):
                    nc.scalar.dma_start(w1v[:, :, q * 256:(q + 1) * 256],
                                        w13[:, :, q * 256:(q + 1) * 256])

            def load_w2():
                for j in range(KF // 2):
                    nc.scalar.dma_start(w2v[:, 2 * j:2 * j + 2, :],
                                        w23[:, 2 * j:2 * j + 2, :])
                nc.scalar.dma_start(
                    wv_tile[:, :], wv.rearrange("(j p) o -> p (j o)", p=128))

            def load_x(c):
                cb, cw = chunks[c]
                x_tile = xpool.tile([128, KD * cw], f16, tag="x", name=f"x_{c}")
                nc.sync.dma_start(
                    x_tile[:, :].rearrange("p (k t) -> p k t", t=cw),
                    xT3[:, :, cb:cb + cw])
                return x_tile

            def fc1(c, x_tile):
                cb, cw = chunks[c]
                h_tile = hpool.tile([128, KF * cw], f16, tag="h", name=f"h_{c}")
                for f in range(KF):
                    ph = ph_pool.tile([128, cw], f32, tag="ph",
                                      name=f"ph_{c}_{f}")
                    for k in range(KD):
                        nc.tensor.matmul(
                            ph[:],
                            w1_tile[:, k * F + f * 128:k * F + (f + 1) * 128],
                            x_tile[:, k * cw:(k + 1) * cw],
                            start=(k == 0), stop=(k == KD - 1))
                    hr = rpool.tile([128, cw], f32, tag="hr",
                                    name=f"hr_{c}_{f}")
                    nc.vector.tensor_scalar_max(hr[:], ph[:], 0.0)
                    nc.scalar.square(h_tile[:, f * cw:(f + 1) * cw], hr[:])
                return h_tile

            def fc2(c, h_tile):
                cb, cw = chunks[c]
                for s in range(cw // 128):
                    yt = ypool.tile([128, D], f32, tag="y", name=f"y_{c}_{s}")
                    for dn in range(D // 512):
                        py = py_pool.tile([128, 512], f32, tag="py",
                                          name=f"py_{c}_{s}_{dn}")
                        for f in range(KF):
                            nc.tensor.matmul(
                                py[:],
                                h_tile[:, f * cw + s * 128:
                                       f * cw + (s + 1) * 128],
                                w2_tile[:, f * D + dn * 512:
                                        f * D + (dn + 1) * 512],
                                start=(f == 0), stop=(f == KF - 1))
                        nc.scalar.mul(yt[:, dn * 512:(dn + 1) * 512], py[:],
                                      wv_tile[:, (cb + s * 128) // 128:
                                              (cb + s * 128) // 128 + 1])
                        nc.sync.dma_start(
                            y[cb + s * 128:cb + (s + 1) * 128,
                              dn * 512:(dn + 1) * 512],
                            yt[:, dn * 512:(dn + 1) * 512])

            def body(_=None):
                L = interleave  # fc1 runs L chunks ahead of fc2
                x_tiles = {}
                h_tiles = {}
                x_tiles[0] = load_x(0)
                x_tiles[1] = load_x(1)
                load_w1()
                load_w2()
                for c in range(2, min(L + 1, NCH)):
                    x_tiles[c] = load_x(c)
                for c in range(L):
                    if c < NCH:
                        h_tiles[c] = fc1(c, x_tiles.pop(c))
                for i in range(NCH):
                    c_ahead = i + L
                    if c_ahead + 1 < NCH:
                        x_tiles[c_ahead + 1] = load_x(c_ahead + 1)
                    if c_ahead < NCH:
                        h_tiles[c_ahead] = fc1(c_ahead, x_tiles.pop(c_ahead))
                    fc2(i, h_tiles.pop(i))

            if repeat == 1:
                body()
            else:
                with tc.For_i(0, repeat, 1) as _i:
                    body(_i)
    nc.compile()
    return nc


def _get_module(name):
    if name not in _CACHE:
        if name == "expert":
            _CACHE[name] = _build_expert_module()
        else:
            raise KeyError(name)
    return _CACHE[name]


def _routing_from_logits(logits):
    """Replicates reference softmax/top-2/normalize in fp32 numpy.

    jax.lax.top_k tie-break (lower index first) == stable argsort on -p.
    """
    logits = logits.astype(np.float32, copy=False)
    m = logits.max(axis=1, keepdims=True)
    p = np.exp(logits - m)
    p = (p / p.sum(axis=1, keepdims=True)).astype(np.float32)
    order = np.argsort(-p, axis=1, kind="stable")
    t1 = order[:, 0].astype(np.int32)
    t2 = order[:, 1].astype(np.int32)
    ar = np.arange(logits.shape[0])
    tv1 = p[ar, t1]
    tv2 = p[ar, t2]
    s = (tv1 + tv2).astype(np.float32)
    w1 = (tv1 / s).astype(np.float32)
    w2 = (tv2 / s).astype(np.float32)
    return t1, t2, w1, w2


def kernel(x, router_w, fc1_w, fc2_w):
    from concourse.bass_utils import run_bass_kernel_spmd

    x = np.ascontiguousarray(np.asarray(x, dtype=np.float32))
    router_w = np.ascontiguousarray(np.asarray(router_w, dtype=np.float32))
    fc1_w = np.asarray(fc1_w, dtype=np.float32)
    fc2_w = np.asarray(fc2_w, dtype=np.float32)

    B, T, D = x.shape
    xf = x.reshape(B * T, D)
    xT = np.ascontiguousarray(xf.T)               # [D, N]

    # --- router on host: fp32 logits + softmax/top-2 dispatch ---
    logits = xf @ router_w.T                      # [N, E] fp32 BLAS
    global _LAST_LOGITS
    _LAST_LOGITS = logits
    t1, t2, w1, w2 = _routing_from_logits(logits)
    idx_e = []
    wv_e = []
    for e in range(N_EXPERTS):
        sel = np.where((t1 == e) | (t2 == e))[0]
        idx_e.append(sel)
        wv_e.append(np.where(t1[sel] == e, w1[sel], w2[sel]).astype(np.float32))

    # --- device launch: expert FFN ---
    nc_e = _get_module("expert")
    w1T_np = [np.ascontiguousarray(fc1_w[e].T).astype(np.float16)
              for e in range(N_EXPERTS)]
    w2T_np = [np.ascontiguousarray(fc2_w[e].T).astype(np.float16)
              for e in range(N_EXPERTS)]
    out = np.zeros((B * T, D), np.float32)
    n_passes = max(1, -(-max(len(s) for s in idx_e) // CAP))
    for p in range(n_passes):  # overflow fallback: extra passes never trigger
        in_maps = []           # for the fixed problem size (max count 2175)
        for e in range(N_EXPERTS):
            sl = idx_e[e][p * CAP:(p + 1) * CAP]
            xg = np.zeros((D, CAP), np.float16)
            xg[:, :len(sl)] = xT[:, sl].astype(np.float16)
            wvg = np.zeros((CAP, 1), np.float32)
            wvg[:len(sl), 0] = wv_e[e][p * CAP:(p + 1) * CAP]
            in_maps.append({"xT": xg, "w1T": w1T_np[e], "w2T": w2T_np[e],
                            "wv": wvg})
        res = run_bass_kernel_spmd(nc_e, in_maps, core_ids=list(range(N_CORES)))
        # host combine (ascending expert order == reference accumulation order)
        for e in range(N_EXPERTS):
            sl = idx_e[e][p * CAP:(p + 1) * CAP]
            out[sl] += res.results[e]["y"][:len(sl)]
    return out.reshape(B, T, D)
